# revision 11
# baseline (speedup 1.0000x reference)
"""Trainium2 Bass kernel for CategoricalEntropyRegLoss.

Math: both loss terms factor so the [B,B] pairwise matrices are never built.
After L2-normalization sq_j = 1 exactly, so the m*sq column collapses into
the mask column (a = M, se = e, Psq = Pbar, Lsq = L):

  S = sum_{jk} m_j m_k feat_dists * target_dists
    = 2*[ M*e - Pbar.Lbar - Fe.F~ + <U,V> ]
  div = -S / (D * M * (M-1))          (D = 2)
  tight*M = M - sum_s ||seg_sum_s||^2 / max(cnt_s,1)

One matmul per core:
  out[1154, 257] = ext_seg^T @ ext_feat
  ext_seg  = [ onehot(code) | LQ | P | 1 | E ]      (B x 1154)
  ext_feat = [ m*fn | m ]                           (B x 257)
Matmul operands and the AllReduce payload are bf16: one-hot/mask/counts
are exact in bf16 (integers <= 256), features round at ~2^-9 rel; the
measured end-to-end rel err is ~4e-4 against a 2e-2 gate. argmax/code
stay fp32 so segment assignment matches the reference. A single 8-core
AllReduce combines the partials; every core runs the cheap epilogue
redundantly.
"""

import numpy as np

B = 4096
FD = 256
C = 32
D = 2
NSEG = C ** D          # 1024
NCORES = 8
RB = B // NCORES       # 512 rows per core
KT = RB // 128         # 4 k-chunks of 128 rows
EF = FD + 1            # 257: [mfn | m]
ES = NSEG + 2 * D * C + 2   # 1154: [onehot | LQ | P | ones | E]
NMT = (ES + 127) // 128     # 10 m-tiles (last has 2 rows)
NST = 2 * D * C + 2         # 130 stats columns

_compiled = {}


def _build_bass():
    from contextlib import ExitStack
    import concourse.bass as bass
    import concourse.bacc as bacc
    import concourse.tile as tile
    from concourse import mybir

    from concourse.tile import add_dep_helper

    f32 = mybir.dt.float32
    bf16 = mybir.dt.bfloat16
    fp16 = mybir.dt.float16
    Alu = mybir.AluOpType
    Act = mybir.ActivationFunctionType
    Ax = mybir.AxisListType

    nc = bacc.Bacc(num_devices=NCORES)

    feat = nc.dram_tensor("features", [RB, FD], f32, kind="ExternalInput")
    targ = nc.dram_tensor("targets", [RB, D * C], f32, kind="ExternalInput")
    maskf = nc.dram_tensor("maskf", [RB, 1], f32, kind="ExternalInput")
    outd = nc.dram_tensor("out", [8], f32, kind="ExternalOutput")

    with ExitStack() as ctx:
        tc = ctx.enter_context(tile.TileContext(nc))
        consts = ctx.enter_context(tc.tile_pool(name="consts", bufs=1))
        work = ctx.enter_context(tc.tile_pool(name="work", bufs=1))
        keep = ctx.enter_context(tc.tile_pool(name="keep", bufs=1))
        psum = ctx.enter_context(tc.tile_pool(name="psum", bufs=1, space="PSUM"))
        dram = ctx.enter_context(tc.tile_pool(name="dram", bufs=1, space="DRAM"))

        # ---------------- constants ----------------
        ones128 = consts.tile([128, 1], f32)
        nc.vector.memset(ones128[:], 1.0)

        # ACT Square-table prefetch: tiny op so the table is resident
        # before the first real Square
        pre = consts.tile([1, 1], f32)
        pre_act = nc.scalar.activation(out=pre[:], in_=ones128[0:1, 0:1],
                                       func=Act.Square)

        # partition-major AllReduce buffer: [p, mt, f] bf16.
        # Slot (p, 9, :) is zero-padded for p>=2.
        inbounce = dram.tile([128, NMT, EF], fp16, name="inbounce")
        outbounce = dram.tile([128, NMT, EF], fp16, name="outbounce",
                              addr_space="Shared")

        # ---- batched input loads spread over queues ----
        tbig = keep.tile([128, KT, D * C], f32, name="tbig")
        nc.scalar.dma_start(
            out=tbig[:], in_=targ[:, :].rearrange("(a p) f -> p a f", p=128))
        mkbig = keep.tile([128, KT, 1], f32, name="mkbig")
        nc.scalar.dma_start(
            out=mkbig[:], in_=maskf[:, :].rearrange("(a p) f -> p a f", p=128))
        # two tiles (not halves of one) so chunk reads only wait their own DMA
        xbig0 = keep.tile([128, 2, FD], f32, name="xbig0")
        nc.sync.dma_start(
            out=xbig0[:],
            in_=feat[0:256, :].rearrange("(a p) f -> p a f", p=128))
        xbig1 = keep.tile([128, 2, FD], f32, name="xbig1")
        nc.gpsimd.dma_start(
            out=xbig1[:],
            in_=feat[256:512, :].rearrange("(a p) f -> p a f", p=128))

        def xchunk(kc):
            return xbig0[:, kc, :] if kc < 2 else xbig1[:, kc - 2, :]

        # iotas after the gpsimd input DMA trigger
        iota1024 = consts.tile([128, NSEG], f32)
        nc.gpsimd.iota(iota1024[:], [[1, NSEG]], channel_multiplier=0,
                       allow_small_or_imprecise_dtypes=True)
        # biota[j] = 32 - j  (for first-argmax via reduce_max)
        biota = consts.tile([128, C], f32)
        nc.gpsimd.iota(biota[:], [[-1, C]], base=C, channel_multiplier=0,
                       allow_small_or_imprecise_dtypes=True)

        # bf16 matmul operands; argmax/code in fp32
        es_oh = [keep.tile([128, NSEG], bf16, name=f"esoh_{kc}")
                 for kc in range(KT)]
        es_st = [keep.tile([128, NST], bf16, name=f"esst_{kc}")
                 for kc in range(KT)]
        ef_b = [keep.tile([128, EF], bf16, name=f"efb_{kc}")
                for kc in range(KT)]

        # ---- ACT phase 1: row sum-of-squares ----
        sqpack = keep.tile([128, KT], f32, name="sqpack")
        scrsq = keep.tile([128, FD], f32, name="scrsq")
        act_chain = [pre_act]
        for kc in range(KT):
            act_chain.append(nc.scalar.activation(
                out=scrsq[:], in_=xchunk(kc), func=Act.Square,
                accum_out=sqpack[:, kc:kc + 1]))
        # ---- ACT phase 2: one Sqrt for all chunks ----
        normpack = keep.tile([128, KT], f32, name="normpack")
        act_chain.append(nc.scalar.sqrt(normpack[:], sqpack[:]))
        nc.vector.tensor_scalar_max(out=normpack[:], in0=normpack[:],
                                    scalar1=1e-12)
        invpack = keep.tile([128, KT], f32, name="invpack")
        nc.vector.reciprocal(invpack[:], normpack[:])
        # minv = m * inv  (fold mask into the normalization scale)
        minvpack = keep.tile([128, KT], f32, name="minvpack")
        nc.vector.tensor_tensor(out=minvpack[:], in0=invpack[:],
                                in1=mkbig[:, :, 0], op=Alu.mult)

        # ---- targets chains (DVE) + Ln (ACT phase 3) ----
        # es_st columns: [0:64 lq | 64:128 p | 128 ones | 129 E]
        t1big = keep.tile([128, KT, D * C], f32, name="t1big")
        nc.vector.tensor_scalar_add(out=t1big[:], in0=tbig[:], scalar1=1e-10)
        invsb = keep.tile([128, KT * D], f32, name="invsb")
        nc.vector.reduce_sum(
            out=invsb[:],
            in_=t1big[:].rearrange("p a (d c) -> p (a d) c", c=C),
            axis=Ax.X)
        nc.vector.reciprocal(invsb[:], invsb[:])
        ln_acts = []
        pts = [work.tile([128, D * C], f32, name=f"pt_{kc}", tag=f"pt_{kc}")
               for kc in range(KT)]
        lqws = [work.tile([128, D * C], f32, name=f"lqw_{kc}", tag=f"lq_{kc}")
                for kc in range(KT)]
        for kc in range(KT):
            st_t = es_st[kc]
            pt = pts[kc]
            lqw = lqws[kc]
            for d_ in range(D):
                nc.vector.tensor_scalar_mul(
                    out=pt[:, C * d_:C * (d_ + 1)],
                    in0=t1big[:, kc, C * d_:C * (d_ + 1)],
                    scalar1=invsb[:, kc * D + d_:kc * D + d_ + 1])
            ln_acts.append(nc.scalar.activation(out=lqw[:], in_=pt[:],
                                                func=Act.Ln))
            nc.vector.tensor_copy(out=st_t[:, 0:D * C], in_=lqw[:])
            nc.vector.tensor_copy(out=st_t[:, D * C:2 * D * C], in_=pt[:])

            # ---- first-argmax per dim, then code = cls0 + 32*cls1 ----
            cls = work.tile([128, D], f32, name=f"cls_{kc}", tag=f"cl_{kc}")
            for d_ in range(D):
                pch = pt[:, C * d_:C * (d_ + 1)]
                mx = work.tile([128, 1], f32, name=f"mx_{kc}_{d_}",
                               tag=f"mx_{kc}_{d_}")
                nc.vector.reduce_max(out=mx[:], in_=pch, axis=Ax.X)
                cand = work.tile([128, C], f32, name=f"cand_{kc}_{d_}",
                                 tag=f"cd_{kc}_{d_}")
                # (p == max) * (32 - idx); reduce_max -> 32 - first_argmax
                nc.vector.scalar_tensor_tensor(
                    out=cand[:], in0=pch, scalar=mx[:], in1=biota[:],
                    op0=Alu.is_equal, op1=Alu.mult)
                mq = work.tile([128, 1], f32, name=f"mq_{kc}_{d_}",
                               tag=f"mq_{kc}_{d_}")
                nc.vector.reduce_max(out=mq[:], in_=cand[:], axis=Ax.X)
                nc.vector.tensor_scalar(
                    out=cls[:, d_:d_ + 1], in0=mq[:], scalar1=-1.0,
                    scalar2=float(C), op0=Alu.mult, op1=Alu.add)
            code = work.tile([128, 1], f32, name=f"code_{kc}", tag=f"co_{kc}")
            nc.vector.tensor_scalar(
                out=code[:], in0=cls[:, 1:2], scalar1=float(C),
                scalar2=cls[:, 0:1], op0=Alu.mult, op1=Alu.add)
            # ---- one-hot (DVE), bf16 out (exact 0/1) ----
            nc.vector.tensor_scalar(
                out=es_oh[kc][:], in0=iota1024[:], scalar1=code[:],
                scalar2=None, op0=Alu.is_equal)

        # ---- ext_feat = [x*(m*inv) | m] bf16 (ACT phase 4) ----
        copy_acts = []
        for kc in range(KT):
            ef_t = ef_b[kc]
            copy_acts.append(nc.scalar.activation(
                out=ef_t[:, 0:FD], in_=xchunk(kc), func=Act.Copy,
                scale=minvpack[:, kc:kc + 1]))
            nc.vector.tensor_copy(out=ef_t[:, FD:FD + 1], in_=mkbig[:, kc, :])

        # E / ones columns
        for kc in range(KT):
            st_t = es_st[kc]
            scr64 = work.tile([128, D * C], f32, name=f"scr64_{kc}",
                              tag=f"s64_{kc}")
            nc.vector.tensor_tensor(out=scr64[:], in0=pts[kc][:],
                                    in1=lqws[kc][:], op=Alu.mult)
            ecol = work.tile([128, 1], f32, name=f"ecol_{kc}",
                             tag=f"ec_{kc}")
            nc.vector.reduce_sum(out=ecol[:], in_=scr64[:], axis=Ax.X)
            nc.vector.tensor_copy(out=st_t[:, NST - 1:NST], in_=ecol[:])
            nc.vector.memset(st_t[:, NST - 2:NST - 1], 1.0)

        # re-prefetch the Square table during the matmul/AR so the
        # epilogue squares don't pay the table reload
        pre2 = consts.tile([1, 1], f32)
        pre2_act = nc.scalar.activation(out=pre2[:], in_=ones128[0:1, 0:1],
                                        func=Act.Square)

        # keep ACT ops grouped by function (avoid act-table reload thrash);
        # table-less Copies run before the Lns so ef is ready sooner
        act_chain = act_chain + copy_acts + ln_acts + [pre2_act]
        for a, b in zip(act_chain[1:], act_chain[:-1]):
            add_dep_helper(a.ins, b.ins, sync=False,
                           reason="act table grouping")

        # ---------------- the one big matmul ----------------
        # separate result tiles per store so no DMA reads a tile that later
        # copies write
        resa = keep.tile([128, 4, EF], fp16, name="resa")
        resb = keep.tile([128, 4, EF], fp16, name="resb")
        resc = keep.tile([128, 2, EF], fp16, name="resc")
        nc.vector.memset(resc[:], 0.0)
        for mt in range(NMT):
            mlo = mt * 128
            msz = min(128, ES - mlo)
            ps = psum.tile([msz, EF], f32, name=f"ps_{mt}", tag=f"ps_{mt % 7}")
            for kc in range(KT):
                if mt < 8:
                    lhsT = es_oh[kc][:, mlo:mlo + msz]
                else:
                    lhsT = es_st[kc][:, mlo - NSEG:mlo - NSEG + msz]
                nc.tensor.matmul(out=ps[:], lhsT=lhsT, rhs=ef_b[kc][:],
                                 start=(kc == 0), stop=(kc == KT - 1))
            if mt < 4:
                nc.vector.tensor_copy(out=resa[:, mt, :], in_=ps[:])
            elif mt < 8:
                nc.vector.tensor_copy(out=resb[:, mt - 4, :], in_=ps[:])
            else:
                nc.vector.tensor_copy(out=resc[0:msz, mt - 8, :], in_=ps[:])
            if mt == 3:
                nc.sync.dma_start(out=inbounce[:, 0:4, :], in_=resa[:])
            elif mt == 7:
                nc.gpsimd.dma_start(out=inbounce[:, 4:8, :], in_=resb[:])
            elif mt == 9:
                nc.scalar.dma_start(out=inbounce[:, 8:10, :], in_=resc[:])

        # ---------------- single AllReduce (bf16) ----------------
        nc.gpsimd.collective_compute(
            "AllReduce", mybir.AluOpType.add,
            replica_groups=[list(range(NCORES))],
            ins=[inbounce.opt()], outs=[outbounce.opt()])

        # ---------------- epilogue (redundant on every core) ----------------
        big0 = keep.tile([128, 4, EF], fp16, name="big0")
        nc.sync.dma_start(out=big0[:], in_=outbounce[:, 0:4, :])
        big1 = keep.tile([128, 4, EF], fp16, name="big1")
        nc.scalar.dma_start(out=big1[:], in_=outbounce[:, 4:8, :])
        last2 = keep.tile([1, EF], fp16, name="last2")
        nc.scalar.dma_start(out=last2[:], in_=outbounce[0:1, 9, :])
        r1 = keep.tile([1, EF], fp16, name="r1")
        nc.scalar.dma_start(out=r1[:], in_=outbounce[1:2, 9, :])

        Z = keep.tile([128, 8], f32, name="Z")
        nc.vector.memset(Z[:], 0.0)
        nrmp = keep.tile([128, 8], f32, name="nrmp")
        cdp = keep.tile([128, 8], f32, name="cdp")
        # segment-center squares on ACT (Square table prefetched above);
        # DVE meanwhile handles counts/stats
        sq_acts = []
        for sl in range(4):
            sq_acts.append(nc.scalar.activation(
                out=scrsq[:], in_=big0[:, sl, 0:FD], func=Act.Square,
                accum_out=nrmp[:, sl:sl + 1]))
        for sl in range(4):
            sq_acts.append(nc.scalar.activation(
                out=scrsq[:], in_=big1[:, sl, 0:FD], func=Act.Square,
                accum_out=nrmp[:, 4 + sl:5 + sl]))
        for a, b in zip(sq_acts, [pre2_act] + sq_acts[:-1]):
            add_dep_helper(a.ins, b.ins, sync=False,
                           reason="act table grouping")
        nc.vector.tensor_scalar_max(out=cdp[:, 0:4], in0=big0[:, :, FD],
                                    scalar1=1.0)
        nc.vector.tensor_scalar_max(out=cdp[:, 4:8], in0=big1[:, :, FD],
                                    scalar1=1.0)
        rcdp = keep.tile([128, 8], f32, name="rcdp")
        nc.vector.reciprocal(rcdp[:], cdp[:])
        termp = keep.tile([128, 8], f32, name="termp")
        nc.vector.tensor_tensor(out=termp[:], in0=nrmp[:], in1=rcdp[:],
                                op=Alu.mult)
        nc.vector.reduce_sum(out=Z[:, 0:1], in_=termp[:], axis=Ax.X)

        # stats m-tile 8: partitions 0:64 = U^T rows, 64:128 = V^T rows
        ut = keep.tile([64, EF], fp16, name="ut")
        nc.sync.dma_start(out=ut[:], in_=outbounce[0:64, 8, :])
        vt = keep.tile([64, EF], fp16, name="vt")
        nc.sync.dma_start(out=vt[:], in_=outbounce[64:128, 8, :])

        scrU = keep.tile([64, FD], f32, name="scrU")
        nc.vector.tensor_tensor(out=scrU[:], in0=ut[:, 0:FD],
                                in1=vt[:, 0:FD], op=Alu.mult)
        nc.vector.reduce_sum(out=Z[0:64, 1:2], in_=scrU[:], axis=Ax.X)
        # Pbar.Lbar: product of the two mask columns
        nc.vector.tensor_tensor(out=Z[0:64, 2:3], in0=vt[:, FD:FD + 1],
                                in1=ut[:, FD:FD + 1], op=Alu.mult)
        scrF = keep.tile([1, FD], f32, name="scrF")
        nc.vector.tensor_tensor(out=scrF[:], in0=last2[:, 0:FD],
                                in1=r1[:, 0:FD], op=Alu.mult)
        nc.vector.reduce_sum(out=Z[0:1, 4:5], in_=scrF[:], axis=Ax.X)  # Fe.F~

        zred = psum.tile([1, 8], f32, name="zred", tag="ps_0")
        nc.tensor.matmul(out=zred[:], lhsT=ones128[:], rhs=Z[:],
                         start=True, stop=True)
        zs = keep.tile([1, 8], f32, name="zs")
        nc.vector.tensor_copy(out=zs[:], in_=zred[:])

        # scalars: M = last2[256] (ones row x m col), e = r1[256]
        Mv = keep.tile([1, 2], f32, name="Mv")
        nc.vector.tensor_copy(out=Mv[0:1, 0:1], in_=last2[0:1, FD:FD + 1])
        nc.vector.tensor_copy(out=Mv[0:1, 1:2], in_=r1[0:1, FD:FD + 1])
        Ms = Mv[0:1, 0:1]
        ev = Mv[0:1, 1:2]
        s_center = zs[0:1, 0:1]
        uv = zs[0:1, 1:2]
        pl = zs[0:1, 2:3]
        fef = zs[0:1, 4:5]

        fin = keep.tile([1, 16], f32, name="fin")
        t_ = lambda i: fin[0:1, i:i + 1]
        # inner = M*e - pl - fef + uv
        nc.vector.tensor_tensor(out=t_(8), in0=Ms, in1=ev, op=Alu.mult)
        nc.vector.tensor_tensor(out=t_(9), in0=t_(8), in1=pl, op=Alu.subtract)
        nc.vector.tensor_tensor(out=t_(10), in0=t_(9), in1=fef, op=Alu.subtract)
        nc.vector.tensor_tensor(out=t_(11), in0=t_(10), in1=uv, op=Alu.add)
        # md = M*(M-1) ; div = -inner/md
        nc.vector.tensor_scalar(out=t_(15), in0=Ms, scalar1=-1.0,
                                scalar2=Ms, op0=Alu.add, op1=Alu.mult)
        nc.vector.reciprocal(t_(15), t_(15))
        nc.vector.tensor_tensor(out=t_(12), in0=t_(11), in1=t_(15), op=Alu.mult)
        nc.vector.tensor_scalar_mul(out=t_(1), in0=t_(12), scalar1=-1.0)
        # tight = 1 - s_center/M
        nc.vector.reciprocal(t_(6), Ms)
        nc.vector.tensor_tensor(out=t_(7), in0=s_center, in1=t_(6), op=Alu.mult)
        nc.vector.tensor_scalar(out=t_(2), in0=t_(7), scalar1=-1.0,
                                scalar2=1.0, op0=Alu.mult, op1=Alu.add)
        # total = 0.1*div + 0.1*tight
        nc.vector.tensor_tensor(out=t_(0), in0=t_(1), in1=t_(2), op=Alu.add)
        nc.vector.tensor_scalar_mul(out=t_(0), in0=t_(0), scalar1=0.1)
        # debug slots
        nc.vector.tensor_copy(out=t_(3), in_=Ms)
        nc.vector.tensor_copy(out=t_(4), in_=ev)
        nc.vector.tensor_copy(out=t_(5), in_=uv)

        nc.sync.dma_start(out=outd[None, :], in_=fin[0:1, 0:8])

    nc.finalize()
    return nc


def _get_compiled():
    if "nc" not in _compiled:
        _compiled["nc"] = _build_bass()
    return _compiled["nc"]


def _make_in_maps(features, targets, mask):
    features = np.ascontiguousarray(np.asarray(features, dtype=np.float32))
    targets = np.ascontiguousarray(np.asarray(targets, dtype=np.float32))
    maskf = np.asarray(mask).astype(np.float32).reshape(B, 1)
    in_maps = []
    for i in range(NCORES):
        sl = slice(i * RB, (i + 1) * RB)
        in_maps.append({
            "features": features[sl],
            "targets": targets[sl],
            "maskf": np.ascontiguousarray(maskf[sl]),
        })
    return in_maps


def kernel(features, targets, mask):
    from concourse.bass_utils import run_bass_kernel_spmd

    nc = _get_compiled()
    in_maps = _make_in_maps(features, targets, mask)
    res = run_bass_kernel_spmd(nc, in_maps, list(range(NCORES)))
    out = res.results[0]["out"]
    total = np.float32(out[0])
    diversity = np.float32(out[1])
    tightness = np.float32(out[2])
    return total, diversity, tightness


# revision 16
# speedup vs baseline: 1.1914x; 1.1914x over previous
"""Trainium2 Bass kernel for CategoricalEntropyRegLoss.

Math: both loss terms factor so the [B,B] pairwise matrices are never built.
After L2-normalization sq_j = 1 exactly, so the m*sq column collapses into
the mask column (a = M, se = e, Psq = Pbar, Lsq = L):

  S = sum_{jk} m_j m_k feat_dists * target_dists
    = 2*[ M*e - Pbar.Lbar - Fe.F~ + <U,V> ]
  div = -S / (D * M * (M-1))          (D = 2)
  tight*M = M - sum_s ||seg_sum_s||^2 / max(cnt_s,1)

One matmul per core:
  out[1154, 257] = ext_seg^T @ ext_feat
  ext_seg  = [ onehot(code) | LQ | P | 1 | E ]      (B x 1154)
  ext_feat = [ m*fn | m ]                           (B x 257)
Matmul operands are bf16, the AllReduce payload fp16 (counts <= 2048
exact; stats rows get fp16's 2^-12 rounding, measured end-to-end rel err
~5e-3 against a 2e-2 gate). argmax/code stay fp32 so segment assignment
matches the reference.

The matmul runs k-chunk-outer in two PSUM waves (A: m-tiles 0-6, B:
7-9) so chunk 0's matmuls start as soon as its one-hot and scaled
features exist, overlapping the remaining preprocessing. argmax runs on
the un-normalized targets (positive per-row scaling preserves argmax),
decoupling it from the normalize chain. A single AllReduce combines the
partials; every core runs the cheap epilogue redundantly.
"""

import numpy as np

B = 4096
FD = 256
C = 32
D = 2
NSEG = C ** D          # 1024
NCORES = 8
RB = B // NCORES       # 512 rows per core
KT = RB // 128         # 4 k-chunks of 128 rows
EF = FD + 1            # 257: [mfn | m]
ES = NSEG + 2 * D * C + 2   # 1154: [onehot | LQ | P | ones | E]
NMT = (ES + 127) // 128     # 10 m-tiles (last has 2 rows)
NST = 2 * D * C + 2         # 130 stats columns
NA = 7                      # wave A m-tiles (0..6); wave B: 7..9

_compiled = {}


def _build_bass():
    from contextlib import ExitStack
    import concourse.bass as bass
    import concourse.bacc as bacc
    import concourse.tile as tile
    from concourse import mybir

    from concourse.tile import add_dep_helper

    f32 = mybir.dt.float32
    bf16 = mybir.dt.bfloat16
    fp16 = mybir.dt.float16
    Alu = mybir.AluOpType
    Act = mybir.ActivationFunctionType
    Ax = mybir.AxisListType

    nc = bacc.Bacc(num_devices=NCORES)

    feat = nc.dram_tensor("features", [RB, FD], f32, kind="ExternalInput")
    targ = nc.dram_tensor("targets", [RB, D * C], f32, kind="ExternalInput")
    maskf = nc.dram_tensor("maskf", [RB, 1], f32, kind="ExternalInput")
    outd = nc.dram_tensor("out", [8], f32, kind="ExternalOutput")

    with ExitStack() as ctx:
        tc = ctx.enter_context(tile.TileContext(nc))
        consts = ctx.enter_context(tc.tile_pool(name="consts", bufs=1))
        work = ctx.enter_context(tc.tile_pool(name="work", bufs=1))
        keep = ctx.enter_context(tc.tile_pool(name="keep", bufs=1))
        psum = ctx.enter_context(tc.tile_pool(name="psum", bufs=1, space="PSUM"))
        dram = ctx.enter_context(tc.tile_pool(name="dram", bufs=1, space="DRAM"))

        # ---------------- constants ----------------
        ones128 = consts.tile([128, 1], f32)
        nc.vector.memset(ones128[:], 1.0)

        # ACT Square-table prefetch before the input DMAs land
        pre = consts.tile([1, 1], f32)
        pre_act = nc.scalar.activation(out=pre[:], in_=ones128[0:1, 0:1],
                                       func=Act.Square)

        # partition-major AllReduce buffer: [p, mt, f] fp16.
        # Slot (p, 9, :) is zero-padded for p>=2.
        inbounce = dram.tile([128, NMT, EF], fp16, name="inbounce")
        outbounce = dram.tile([128, NMT, EF], fp16, name="outbounce",
                              addr_space="Shared")

        # ---- batched input loads spread over queues ----
        tbig = keep.tile([128, KT, D * C], f32, name="tbig")
        nc.scalar.dma_start(
            out=tbig[:], in_=targ[:, :].rearrange("(a p) f -> p a f", p=128))
        mkbig = keep.tile([128, KT, 1], f32, name="mkbig")
        nc.scalar.dma_start(
            out=mkbig[:], in_=maskf[:, :].rearrange("(a p) f -> p a f", p=128))
        xbig0 = keep.tile([128, 2, FD], f32, name="xbig0")
        nc.sync.dma_start(
            out=xbig0[:],
            in_=feat[0:256, :].rearrange("(a p) f -> p a f", p=128))
        xbig1 = keep.tile([128, 2, FD], f32, name="xbig1")
        nc.gpsimd.dma_start(
            out=xbig1[:],
            in_=feat[256:512, :].rearrange("(a p) f -> p a f", p=128))

        def xchunk(kc):
            return xbig0[:, kc, :] if kc < 2 else xbig1[:, kc - 2, :]

        # iotas after the gpsimd input DMA trigger
        iota1024 = consts.tile([128, NSEG], f32)
        nc.gpsimd.iota(iota1024[:], [[1, NSEG]], channel_multiplier=0,
                       allow_small_or_imprecise_dtypes=True)
        # biota[j] = 32 - j  (for first-argmax via reduce_max)
        biota = consts.tile([128, C], f32)
        nc.gpsimd.iota(biota[:], [[-1, C]], base=C, channel_multiplier=0,
                       allow_small_or_imprecise_dtypes=True)

        # bf16 matmul operands; argmax/code in fp32
        es_oh = [keep.tile([128, NSEG], bf16, name=f"esoh_{kc}")
                 for kc in range(KT)]
        es_st = keep.tile([128, KT, NST], bf16, name="esst")
        ef_b = keep.tile([128, KT, EF], bf16, name="efb")

        # ---- argmax path straight off the raw targets (DVE only):
        # argmax(p) == argmax(t + 1e-10) since normalization is a
        # positive per-(row,dim) scale
        t1big = keep.tile([128, KT, D * C], f32, name="t1big")
        nc.vector.tensor_scalar_add(out=t1big[:], in0=tbig[:], scalar1=1e-10)
        t1v = t1big[:].rearrange("p a (d c) -> p (a d) c", c=C)
        mxall = keep.tile([128, KT * D], f32, name="mxall")
        nc.vector.reduce_max(out=mxall[:], in_=t1v, axis=Ax.X)
        candall = keep.tile([128, KT * D, C], f32, name="candall")
        for g in range(KT * D):
            # (t == max) * (32 - idx); reduce_max -> 32 - first_argmax
            nc.vector.scalar_tensor_tensor(
                out=candall[:, g, :], in0=t1v[:, g, :],
                scalar=mxall[:, g:g + 1], in1=biota[:],
                op0=Alu.is_equal, op1=Alu.mult)
        mqall = keep.tile([128, KT * D], f32, name="mqall")
        nc.vector.reduce_max(out=mqall[:], in_=candall[:], axis=Ax.X)
        clsall = keep.tile([128, KT * D], f32, name="clsall")
        nc.vector.tensor_scalar(out=clsall[:], in0=mqall[:], scalar1=-1.0,
                                scalar2=float(C), op0=Alu.mult, op1=Alu.add)
        clsv = clsall[:].rearrange("p (a two) -> p a two", two=2)
        codeall = keep.tile([128, KT], f32, name="codeall")
        # code = 32*cls1 + cls0
        nc.vector.scalar_tensor_tensor(
            out=codeall[:], in0=clsv[:, :, 1], scalar=float(C),
            in1=clsv[:, :, 0], op0=Alu.mult, op1=Alu.add)
        for kc in range(KT):
            nc.vector.tensor_scalar(
                out=es_oh[kc][:], in0=iota1024[:],
                scalar1=codeall[:, kc:kc + 1],
                scalar2=None, op0=Alu.is_equal)

        # ---- probs path: pall = t1 * invs, lqall = Ln(pall) ----
        invsb = keep.tile([128, KT * D], f32, name="invsb")
        nc.vector.reduce_sum(out=invsb[:], in_=t1v, axis=Ax.X)
        nc.vector.reciprocal(invsb[:], invsb[:])
        pall = keep.tile([128, KT * D, C], f32, name="pall")
        for g in range(KT * D):
            nc.vector.tensor_scalar_mul(
                out=pall[:, g, :], in0=t1v[:, g, :],
                scalar1=invsb[:, g:g + 1])
        lqall = keep.tile([128, KT * D, C], f32, name="lqall")
        ln_act = nc.scalar.activation(out=lqall[:], in_=pall[:], func=Act.Ln)

        # ---- ACT phase 1: row sum-of-squares, then Rsqrt ----
        sqpack = keep.tile([128, KT], f32, name="sqpack")
        scrsq = keep.tile([128, FD], f32, name="scrsq")
        act_chain = [pre_act]
        for kc in range(KT):
            act_chain.append(nc.scalar.activation(
                out=scrsq[:], in_=xchunk(kc), func=Act.Square,
                accum_out=sqpack[:, kc:kc + 1]))
        normpack = keep.tile([128, KT], f32, name="normpack")
        act_chain.append(nc.scalar.sqrt(normpack[:], sqpack[:]))
        nc.vector.tensor_scalar_max(out=normpack[:], in0=normpack[:],
                                    scalar1=1e-12)
        invpack = keep.tile([128, KT], f32, name="invpack")
        nc.vector.reciprocal(invpack[:], normpack[:])
        # minv = m / ||x||  (fold mask into the normalization scale)
        minvpack = keep.tile([128, KT], f32, name="minvpack")
        nc.vector.tensor_tensor(out=minvpack[:], in0=invpack[:],
                                in1=mkbig[:, :, 0], op=Alu.mult)

        # ---- ext_feat = [x*(m/||x||) | m] bf16 ----
        for kc in range(KT):
            act_chain.append(nc.scalar.activation(
                out=ef_b[:, kc, 0:FD], in_=xchunk(kc), func=Act.Copy,
                scale=minvpack[:, kc:kc + 1]))
        nc.vector.tensor_copy(out=ef_b[:, :, FD:FD + 1], in_=mkbig[:])

        # ---- stats columns: [lq | p | ones | E] bf16 ----
        nc.vector.tensor_copy(
            out=es_st[:, :, 0:D * C],
            in_=lqall[:].rearrange("p (a d) c -> p a (d c)", d=D))
        nc.vector.tensor_copy(
            out=es_st[:, :, D * C:2 * D * C],
            in_=pall[:].rearrange("p (a d) c -> p a (d c)", d=D))
        scrall = keep.tile([128, KT * D, C], f32, name="scrall")
        nc.vector.tensor_tensor(out=scrall[:], in0=pall[:], in1=lqall[:],
                                op=Alu.mult)
        ecolall = keep.tile([128, KT * D], f32, name="ecolall")
        nc.vector.reduce_sum(out=ecolall[:], in_=scrall[:], axis=Ax.X)
        ecol2 = keep.tile([128, KT], f32, name="ecol2")
        nc.vector.reduce_sum(
            out=ecol2[:],
            in_=ecolall[:].rearrange("p (a d) -> p a d", d=D),
            axis=Ax.X)
        nc.vector.tensor_copy(out=es_st[:, :, NST - 1:NST],
                              in_=ecol2[:].rearrange("p a -> p a ()"))
        nc.vector.memset(es_st[:, :, NST - 2:NST - 1], 1.0)

        # re-prefetch the Square table for the epilogue squares
        pre2 = consts.tile([1, 1], f32)
        pre2_act = nc.scalar.activation(out=pre2[:], in_=ones128[0:1, 0:1],
                                        func=Act.Square)

        # ACT issue order: prefetch, squares, rsqrt, (tableless) copies,
        # Ln, square-prefetch — exactly 4 table loads total
        act_chain = act_chain + [ln_act, pre2_act]
        for a, b in zip(act_chain[1:], act_chain[:-1]):
            add_dep_helper(a.ins, b.ins, sync=False,
                           reason="act table grouping")

        # ---------------- the big matmul: kc-outer, two PSUM waves ------
        resa = keep.tile([128, NA, EF], fp16, name="resa")
        resb = keep.tile([128, NMT - NA, EF], fp16, name="resb")
        nc.vector.memset(resb[:, 2, :], 0.0)

        psA = [psum.tile([128, EF], f32, name=f"psA_{mt}", tag=f"ps_{mt}")
               for mt in range(NA)]
        for kc in range(KT):
            for mt in range(NA):
                nc.tensor.matmul(out=psA[mt][:],
                                 lhsT=es_oh[kc][:, mt * 128:(mt + 1) * 128],
                                 rhs=ef_b[:, kc, :],
                                 start=(kc == 0), stop=(kc == KT - 1))
        for mt in range(NA):
            if mt < 4:
                nc.vector.tensor_copy(out=resa[:, mt, :], in_=psA[mt][:])
            else:
                nc.scalar.activation(out=resa[:, mt, :], in_=psA[mt][:],
                                     func=Act.Copy)
        nc.sync.dma_start(out=inbounce[:, 0:NA, :], in_=resa[:])

        # psB reuses the banks of the earliest-cast wave-A tiles
        psB = [psum.tile([msz, EF], f32, name=f"psB_{i}",
                         tag=f"ps_{[7, 0, 1][i]}")
               for i, msz in enumerate([128, 128, 2])]
        for kc in range(KT):
            nc.tensor.matmul(out=psB[0][:],
                             lhsT=es_oh[kc][:, 7 * 128:8 * 128],
                             rhs=ef_b[:, kc, :],
                             start=(kc == 0), stop=(kc == KT - 1))
            nc.tensor.matmul(out=psB[1][:],
                             lhsT=es_st[:, kc, 0:128],
                             rhs=ef_b[:, kc, :],
                             start=(kc == 0), stop=(kc == KT - 1))
            nc.tensor.matmul(out=psB[2][:],
                             lhsT=es_st[:, kc, 128:130],
                             rhs=ef_b[:, kc, :],
                             start=(kc == 0), stop=(kc == KT - 1))
        nc.vector.tensor_copy(out=resb[:, 0, :], in_=psB[0][:])
        nc.vector.tensor_copy(out=resb[:, 1, :], in_=psB[1][:])
        nc.vector.tensor_copy(out=resb[0:2, 2, :], in_=psB[2][:])
        nc.scalar.dma_start(out=inbounce[:, NA:NMT, :], in_=resb[:])

        # ---------------- single AllReduce (fp16) ----------------
        nc.gpsimd.collective_compute(
            "AllReduce", mybir.AluOpType.add,
            replica_groups=[list(range(NCORES))],
            ins=[inbounce.opt()], outs=[outbounce.opt()])

        # ---------------- epilogue (redundant on every core) ------------
        bigall = keep.tile([128, 8, EF], fp16, name="bigall")
        nc.sync.dma_start(out=bigall[:], in_=outbounce[:, 0:8, :])
        # stats slot 8: U rows on partitions 0:64, V rows on 64:128 ->
        # fold to [64, 2, EF] so U/V pairs share a partition
        uv2 = keep.tile([64, 2, EF], fp16, name="uv2")
        nc.scalar.dma_start(
            out=uv2[:],
            in_=outbounce[:, 8, :].rearrange("(two r) f -> r two f", two=2))
        tail2 = keep.tile([1, 2, EF], fp16, name="tail2")
        nc.scalar.dma_start(out=tail2[:, 0, :], in_=outbounce[0:1, 9, :])
        nc.scalar.dma_start(out=tail2[:, 1, :], in_=outbounce[1:2, 9, :])

        Z = keep.tile([128, 8], f32, name="Z")
        nc.vector.memset(Z[:], 0.0)
        nrmp = keep.tile([128, 8], f32, name="nrmp")
        # segment-center squares on ACT (table prefetched); DVE does counts
        sq_acts = []
        for sl in range(8):
            sq_acts.append(nc.scalar.activation(
                out=scrsq[:], in_=bigall[:, sl, 0:FD], func=Act.Square,
                accum_out=nrmp[:, sl:sl + 1]))
        for a, b in zip(sq_acts, [pre2_act] + sq_acts[:-1]):
            add_dep_helper(a.ins, b.ins, sync=False,
                           reason="act table grouping")
        cdp = keep.tile([128, 8], f32, name="cdp")
        nc.vector.tensor_scalar_max(out=cdp[:], in0=bigall[:, :, FD],
                                    scalar1=1.0)
        rcdp = keep.tile([128, 8], f32, name="rcdp")
        nc.vector.reciprocal(rcdp[:], cdp[:])
        termp = keep.tile([128, 8], f32, name="termp")
        nc.vector.tensor_tensor(out=termp[:], in0=nrmp[:], in1=rcdp[:],
                                op=Alu.mult)
        nc.vector.reduce_sum(out=Z[:, 0:1], in_=termp[:], axis=Ax.X)

        scrU = keep.tile([64, FD], f32, name="scrU")
        nc.vector.tensor_tensor(out=scrU[:], in0=uv2[:, 0, 0:FD],
                                in1=uv2[:, 1, 0:FD], op=Alu.mult)
        nc.vector.reduce_sum(out=Z[0:64, 1:2], in_=scrU[:], axis=Ax.X)
        # Pbar.Lbar: product of the two mask columns
        nc.vector.tensor_tensor(out=Z[0:64, 2:3],
                                in0=uv2[:, 0, FD:FD + 1],
                                in1=uv2[:, 1, FD:FD + 1], op=Alu.mult)
        scrF = keep.tile([1, FD], f32, name="scrF")
        nc.vector.tensor_tensor(out=scrF[:], in0=tail2[:, 0, 0:FD],
                                in1=tail2[:, 1, 0:FD], op=Alu.mult)
        nc.vector.reduce_sum(out=Z[0:1, 4:5], in_=scrF[:], axis=Ax.X)  # Fe.F~

        zred = psum.tile([1, 8], f32, name="zred", tag="ps_3")
        nc.tensor.matmul(out=zred[:], lhsT=ones128[:], rhs=Z[:],
                         start=True, stop=True)
        zs = keep.tile([1, 8], f32, name="zs")
        nc.vector.tensor_copy(out=zs[:], in_=zred[:])

        # scalars: M = tail2[0,0,256] (ones row x m col), e = tail2[0,1,256]
        Mv = keep.tile([1, 2], f32, name="Mv")
        nc.vector.tensor_copy(out=Mv[:], in_=tail2[0:1, :, FD])
        Ms = Mv[0:1, 0:1]
        ev = Mv[0:1, 1:2]
        s_center = zs[0:1, 0:1]
        uv = zs[0:1, 1:2]
        pl = zs[0:1, 2:3]
        fef = zs[0:1, 4:5]

        fin = keep.tile([1, 16], f32, name="fin")
        t_ = lambda i: fin[0:1, i:i + 1]
        # inner = M*e - pl - fef + uv
        nc.vector.tensor_tensor(out=t_(8), in0=Ms, in1=ev, op=Alu.mult)
        nc.vector.tensor_tensor(out=t_(9), in0=t_(8), in1=pl, op=Alu.subtract)
        nc.vector.tensor_tensor(out=t_(10), in0=t_(9), in1=fef, op=Alu.subtract)
        nc.vector.tensor_tensor(out=t_(11), in0=t_(10), in1=uv, op=Alu.add)
        # md = M*(M-1) ; div = -inner/md
        nc.vector.tensor_scalar(out=t_(15), in0=Ms, scalar1=-1.0,
                                scalar2=Ms, op0=Alu.add, op1=Alu.mult)
        nc.vector.reciprocal(t_(15), t_(15))
        nc.vector.tensor_tensor(out=t_(12), in0=t_(11), in1=t_(15), op=Alu.mult)
        nc.vector.tensor_scalar_mul(out=t_(1), in0=t_(12), scalar1=-1.0)
        # tight = 1 - s_center/M
        nc.vector.reciprocal(t_(6), Ms)
        nc.vector.tensor_tensor(out=t_(7), in0=s_center, in1=t_(6), op=Alu.mult)
        nc.vector.tensor_scalar(out=t_(2), in0=t_(7), scalar1=-1.0,
                                scalar2=1.0, op0=Alu.mult, op1=Alu.add)
        # total = 0.1*div + 0.1*tight
        nc.vector.tensor_tensor(out=t_(0), in0=t_(1), in1=t_(2), op=Alu.add)
        nc.vector.tensor_scalar_mul(out=t_(0), in0=t_(0), scalar1=0.1)
        # debug slots
        nc.vector.tensor_copy(out=t_(3), in_=Ms)
        nc.vector.tensor_copy(out=t_(4), in_=ev)
        nc.vector.tensor_copy(out=t_(5), in_=uv)

        nc.sync.dma_start(out=outd[None, :], in_=fin[0:1, 0:8])

    nc.finalize()
    return nc


def _get_compiled():
    if "nc" not in _compiled:
        _compiled["nc"] = _build_bass()
    return _compiled["nc"]


def _make_in_maps(features, targets, mask):
    features = np.ascontiguousarray(np.asarray(features, dtype=np.float32))
    targets = np.ascontiguousarray(np.asarray(targets, dtype=np.float32))
    maskf = np.asarray(mask).astype(np.float32).reshape(B, 1)
    in_maps = []
    for i in range(NCORES):
        sl = slice(i * RB, (i + 1) * RB)
        in_maps.append({
            "features": features[sl],
            "targets": targets[sl],
            "maskf": np.ascontiguousarray(maskf[sl]),
        })
    return in_maps


def kernel(features, targets, mask):
    from concourse.bass_utils import run_bass_kernel_spmd

    nc = _get_compiled()
    in_maps = _make_in_maps(features, targets, mask)
    res = run_bass_kernel_spmd(nc, in_maps, list(range(NCORES)))
    out = res.results[0]["out"]
    total = np.float32(out[0])
    diversity = np.float32(out[1])
    tightness = np.float32(out[2])
    return total, diversity, tightness
